# revision 1
# baseline (speedup 1.0000x reference)
"""Complex-valued dot-product attention (B=4, S=4096, D=64) on 8 TRN2 cores.

Harness entry: kernel(**inputs) -> np.ndarray [2, 4, 4096, 64] fp32.

Math (per batch): s = (q_re + i q_im)(k_re + i k_im)^T / 8,
w = softmax(|s|, axis=keys), out = stack(w @ v_re, w @ v_im).

Sharding: core c = (batch b = c//2, key half h = c%2). Each core computes a
partial softmax (flash-style) over its 2048 keys for all 4096 queries:
  OT = sum_k E^T [v_re|v_im],  R = sum_k E^T,  E = exp(|s|/8 - 6)
and the gather step combines o = (OT_0 + OT_1) / (R_0 + R_1) exactly (the
-6 bias is a shared constant so no per-shard max bookkeeping is needed).

Device kernel (per core): scores are built transposed, s^T [k_part, q_free],
via fp16 matmuls contracting 128 re/im-concatenated channels (Q/K packed
d-major on the host as part of sharding). Per k-tile: ACT Square drains
s_im^T from PSUM while a custom fused DVE op (out = in0^2 + in1) adds
s_re^T^2 in place in an fp16 staging buffer; |s| is then taken either by
ACT Sqrt (iter 0) or by a custom DVE Newton step from an int16-magic rsqrt
seed (iters 1-3) -- balancing the Scalar and Vector engines; exp runs in
place on ACT; the softmax row-sum (ones-matmul) and PV matmuls stream E^T
back through the tensor engine. Everything is emitted software-pipelined at
~0.5-2us instruction granularity so no engine FIFO stalls the others; the
ACT table never thrashes (2 loads total).
"""

import numpy as np

import concourse.bacc as bacc
import concourse.bass as bass
import concourse.mybir as mybir
import concourse.tile as tile


import numpy as np

import concourse.bacc as bacc
import concourse.bass as bass
import concourse.mybir as mybir
import concourse.tile as tile

F16 = mybir.dt.float16
F32 = mybir.dt.float32
I16 = mybir.dt.int16
AF = mybir.ActivationFunctionType

D = 64
SCALE = 1.0 / np.sqrt(np.float32(D))
EXP_BIAS = -6.0
RSQRT_MAGIC = 22970.0
NR_A = 1.5
NR_B = 0.499

_OPS = {}


def _register(name, make_spec):
    from concourse import dve_ops
    from concourse.dve_ops import DveOp
    from concourse.dve_spec import lower, _has_src1
    from concourse.dve_uop import DveOpSpec

    if name in _OPS:
        return _OPS[name]
    for op in dve_ops.OPS:
        if op.name == name:
            _OPS[name] = op
            return op
    spec = make_spec()
    row = dve_ops._CUSTOM_DVE_ROW_BASE + len(dve_ops.OPS)
    assert row < 0x20
    dve_ops._SUB_OPCODE_FOR_NAME[name] = row
    shas = {}
    for ver in ("v3", "v4"):
        ds = DveOpSpec(
            name=name, opcode=row, uops=lower(spec, ver=ver), rd1_en=_has_src1(spec)
        )
        shas[ver] = ds.sha(ver)
    op = DveOp(name, spec, subdim=False, uops_sha=shas)
    dve_ops.OPS.append(op)
    dve_ops.CUSTOM_DVE_SPECS[name] = spec
    _OPS[name] = op
    return op


def register_sumsq():
    from concourse.dve_spec import Spec, Src0, Src1, sq

    return _register(
        "SUMSQ_ANT",
        lambda: Spec(
            body=sq(Src0) + Src1,
            reference=lambda in0, in1, s0, s1, imm2: (
                in0.astype(np.float32) ** 2 + in1.astype(np.float32)
            ),
        ),
    )


def register_sqrt_nr():
    from concourse.dve_spec import Spec, Src0, Src1, C0, C1

    def ref(in0, in1, s0, s1, imm2):
        v = in0.astype(np.float32)
        r0 = in1.astype(np.float32)
        return v * r0 * (s0 - s1 * v * r0 * r0)

    return _register(
        "SQRT_NR_ANT",
        lambda: Spec(
            body=Src0 * (Src1 * (C0 - (Src0 * (Src1 * Src1)) * C1)),
            reference=ref,
        ),
    )


def build_kernel(SQ=4096, SK=2048, CH=512, act_sqrt_iters=(0,)):
    sumsq = register_sumsq()
    sqrt_nr = register_sqrt_nr()
    KT = SK // 128
    NITER = SQ // (2 * CH)
    W = 2 * CH
    REG = KT * W

    nc = bacc.Bacc("TRN2", target_bir_lowering=False)
    qc_d = nc.dram_tensor("qc", [128, SQ], F16, kind="ExternalInput")
    kc1_d = nc.dram_tensor("kc1", [128, SK], F16, kind="ExternalInput")
    kc2_d = nc.dram_tensor("kc2", [128, SK], F16, kind="ExternalInput")
    vc_d = nc.dram_tensor("vc", [KT, 128, 128], F16, kind="ExternalInput")
    ot_d = nc.dram_tensor("ot", [128, SQ], F32, kind="ExternalOutput")
    r_d = nc.dram_tensor("r", [1, SQ], F32, kind="ExternalOutput")

    with tile.TileContext(nc) as tc:
        with (
            tc.tile_pool(name="singles", bufs=1) as singles,
            tc.tile_pool(name="stage", bufs=2) as stage,
            tc.tile_pool(name="seedp", bufs=2) as seedp,
            tc.tile_pool(name="outp", bufs=2) as outp,
            tc.tile_pool(name="ps_re", bufs=2, space="PSUM") as ps_re_pool,
            tc.tile_pool(name="ps_im", bufs=2, space="PSUM") as ps_im_pool,
            tc.tile_pool(name="ps_acc", bufs=1, space="PSUM") as ps_acc,
        ):
            qc = singles.tile([128, SQ], F16)
            kc1 = singles.tile([128, SK], F16)
            kc2 = singles.tile([128, SK], F16)
            vc = singles.tile([128, KT * 128], F16)
            ones = singles.tile([128, 1], F16)
            for i in range(4):
                ksl = slice(i * (SK // 4), (i + 1) * (SK // 4))
                nc.sync.dma_start(kc1[:, ksl], kc1_d.ap()[:, ksl])
                nc.sync.dma_start(kc2[:, ksl], kc2_d.ap()[:, ksl])
                qsl = slice(i * (SQ // 4), (i + 1) * (SQ // 4))
                nc.sync.dma_start(qc[:, qsl], qc_d.ap()[:, qsl])
            for kt in range(KT):
                nc.sync.dma_start(vc[:, kt * 128 : (kt + 1) * 128], vc_d.ap()[kt])
            nc.any.memset(ones[:], 1.0)
            exp_bias = singles.tile([128, 1], F32)
            nc.any.memset(exp_bias[:], EXP_BIAS)

            state = {}

            def emit_a_kt(qi, kt, v_buf):
                k_sl = slice(kt * 128, (kt + 1) * 128)
                res, ims = [], []
                for half in range(2):
                    res.append(ps_re_pool.tile([128, CH], F32, tag="re",
                                               name=f"re_{qi}_{kt}_{half}"))
                    ims.append(ps_im_pool.tile([128, CH], F32, tag="im",
                                               name=f"im_{qi}_{kt}_{half}"))
                for half in range(2):
                    q0 = qi * W + half * CH
                    nc.tensor.matmul(ims[half][:], kc2[:, k_sl],
                                     qc[:, q0 : q0 + CH], start=True, stop=True)
                for half in range(2):
                    q0 = qi * W + half * CH
                    nc.tensor.matmul(res[half][:], kc1[:, k_sl],
                                     qc[:, q0 : q0 + CH], start=True, stop=True)
                for half in range(2):
                    reg = slice(kt * W + half * CH, kt * W + (half + 1) * CH)
                    nc.scalar.activation(v_buf[:, reg], ims[half][:], AF.Square)
                    nc.vector._custom_dve(
                        sumsq, out=v_buf[:, reg], in0=res[half][:], in1=v_buf[:, reg]
                    )

            def emit_sqrt_pair(qi, kt0):
                v_buf = state[qi]["v_buf"]
                sl = slice(kt0 * W, (kt0 + 2) * W)
                nc.scalar.activation(v_buf[:, sl], v_buf[:, sl], AF.Sqrt)

            def emit_nr_pair(qi, kt):
                v_buf = state[qi]["v_buf"]
                sl = slice(kt * W, (kt + 2) * W)
                seed = seedp.tile([128, 2 * W], F16, tag="seed", name=f"seed_{qi}_{kt}")
                nc.vector.tensor_scalar(
                    seed[:].bitcast(I16), v_buf[:, sl].bitcast(I16),
                    -0.5, RSQRT_MAGIC, mybir.AluOpType.mult, mybir.AluOpType.add,
                )
                nc.vector._custom_dve(
                    sqrt_nr, out=v_buf[:, sl], in0=v_buf[:, sl], in1=seed[:],
                    s0=NR_A, s1=NR_B,
                )

            def emit_exp_pair(qi, kt0):
                v_buf = state[qi]["v_buf"]
                sl = slice(kt0 * W, (kt0 + 2) * W)
                nc.scalar.activation(
                    v_buf[:, sl], v_buf[:, sl], AF.Exp,
                    scale=float(SCALE), bias=exp_bias[:],
                )

            def emit_d_kt(qi, kt):
                st = state[qi]
                if "ps_o" not in st:
                    st["ps_o"] = [
                        ps_acc.tile([128, CH], F32, tag="oA", name=f"ps_oA_{qi}"),
                        ps_acc.tile([128, CH], F32, tag="oB", name=f"ps_oB_{qi}"),
                    ]
                    st["ps_r"] = [
                        ps_acc.tile([1, CH], F32, tag="rA", name=f"ps_rA_{qi}"),
                        ps_acc.tile([1, CH], F32, tag="rB", name=f"ps_rB_{qi}"),
                    ]
                v_buf = st["v_buf"]
                for half in range(2):
                    e_half = v_buf[:, kt * W + half * CH : kt * W + (half + 1) * CH]
                    nc.tensor.matmul(
                        st["ps_r"][half][:], ones[:], e_half,
                        start=(kt == 0), stop=(kt == KT - 1),
                    )
                    nc.tensor.matmul(
                        st["ps_o"][half][:],
                        vc[:, kt * 128 : (kt + 1) * 128], e_half,
                        start=(kt == 0), stop=(kt == KT - 1),
                    )

            def emit_out(qi):
                st = state.pop(qi)
                o_sb = outp.tile([128, W], F32, tag="o_sb", name=f"osb{qi}")
                r_sb = outp.tile([1, W], F32, tag="r_sb", name=f"rsb{qi}")
                nc.vector.tensor_copy(o_sb[:, 0:CH], st["ps_o"][0][:])
                nc.vector.tensor_copy(o_sb[:, CH : 2 * CH], st["ps_o"][1][:])
                nc.vector.tensor_copy(r_sb[:, 0:CH], st["ps_r"][0][:])
                nc.vector.tensor_copy(r_sb[:, CH : 2 * CH], st["ps_r"][1][:])
                nc.sync.dma_start(ot_d.ap()[:, qi * W : (qi + 1) * W], o_sb[:])
                nc.sync.dma_start(r_d.ap()[:, qi * W : (qi + 1) * W], r_sb[:])

            for qi in range(NITER + 1):
                prev = qi - 1
                prev_act = prev in act_sqrt_iters if prev >= 0 else False

                if qi < NITER:
                    v_buf = stage.tile([128, REG], F16, tag="v_buf",
                                       name=f"vbuf_{qi}")
                    state[qi] = {"v_buf": v_buf}
                    for kt in range(KT):
                        emit_a_kt(qi, kt, v_buf)
                        if qi not in act_sqrt_iters and kt >= 2 and kt % 2 == 0:
                            emit_nr_pair(qi, kt - 2)
                        if prev >= 0:
                            if prev_act:
                                # kts 0..KT/2-1: sqrt pairs; KT/2..: exp pairs + D
                                if kt < KT // 2:
                                    emit_sqrt_pair(prev, 2 * kt)
                                else:
                                    k2 = 2 * (kt - KT // 2)
                                    emit_exp_pair(prev, k2)
                                    if kt >= KT // 2 + 1:
                                        emit_d_kt(prev, k2 - 2)
                                        emit_d_kt(prev, k2 - 1)
                            else:
                                if kt % 2 == 0:
                                    emit_exp_pair(prev, kt)
                                elif kt >= 3:
                                    emit_d_kt(prev, kt - 3)
                                    emit_d_kt(prev, kt - 2)
                    if qi not in act_sqrt_iters:
                        emit_nr_pair(qi, KT - 2)
                    if prev >= 0:
                        if prev_act:
                            for dk in range(KT - 2, KT):
                                emit_d_kt(prev, dk)
                        else:
                            for dk in range(KT - 2, KT):
                                emit_d_kt(prev, dk)
                        emit_out(prev)
                else:
                    if prev_act:
                        for kt0 in range(0, KT, 2):
                            emit_sqrt_pair(prev, kt0)
                    for kt0 in range(0, KT, 2):
                        emit_exp_pair(prev, kt0)
                        emit_d_kt(prev, kt0)
                        emit_d_kt(prev, kt0 + 1)
                    emit_out(prev)

    nc.compile()
    return nc


# ---------------------------------------------------------------- host packing
def pack_core(q_re, q_im, k_re, k_im, v_re, v_im):
    SK = k_re.shape[0]
    KT = SK // 128
    qc = np.concatenate([q_re.T, q_im.T], axis=0).astype(np.float16)
    kc1 = np.concatenate([k_re.T, -k_im.T], axis=0).astype(np.float16)
    kc2 = np.concatenate([k_im.T, k_re.T], axis=0).astype(np.float16)
    vc = np.concatenate([v_re, v_im], axis=1).astype(np.float16).reshape(KT, 128, 128)
    return {"qc": np.ascontiguousarray(qc), "kc1": np.ascontiguousarray(kc1),
            "kc2": np.ascontiguousarray(kc2), "vc": np.ascontiguousarray(vc)}


def ref_core_partial(q_re, q_im, k_re, k_im, v_re, v_im):
    s_re = (q_re @ k_re.T - q_im @ k_im.T) * SCALE
    s_im = (q_re @ k_im.T + q_im @ k_re.T) * SCALE
    m = np.sqrt(s_re * s_re + s_im * s_im)
    e = np.exp(m + EXP_BIAS)
    ot = np.concatenate([e @ v_re, e @ v_im], axis=1).T
    return ot, e.sum(axis=1)


def combine_host(parts):
    ot = sum(p[0].astype(np.float64) for p in parts)
    r = sum(p[1].reshape(-1).astype(np.float64) for p in parts)
    o = (ot / r[None, :]).astype(np.float32)
    return np.stack([o[0:D].T, o[D : 2 * D].T], axis=0)


# ---------------------------------------------------------------- harness entry
B, S = 4, 4096
SK_HALF = 2048
_CACHE = {}


def _get_nc():
    if "nc" not in _CACHE:
        _CACHE["nc"] = build_kernel()
    return _CACHE["nc"]


def kernel(q_re, q_im, k_re, k_im, v_re, v_im, _trace=False):
    from concourse import bass_utils

    arrs = [np.asarray(a, dtype=np.float32)
            for a in (q_re, q_im, k_re, k_im, v_re, v_im)]
    assert arrs[0].shape == (B, S, D)

    nc = _get_nc()
    maps = []
    for c in range(8):
        b, h = c // 2, c % 2
        ks = slice(h * SK_HALF, (h + 1) * SK_HALF)
        maps.append(pack_core(
            arrs[0][b], arrs[1][b],
            arrs[2][b, ks], arrs[3][b, ks],
            arrs[4][b, ks], arrs[5][b, ks]))
    res = None
    last_exc = None
    for attempt in range(3):
        try:
            res = bass_utils.run_bass_kernel_spmd(
                nc, maps, core_ids=list(range(8)), trace=_trace)
            break
        except Exception as e:  # transient device wedge: retry untraced
            last_exc = e
            _trace = False
    if res is None:
        raise last_exc
    out = np.empty((2, B, S, D), dtype=np.float32)
    for b in range(B):
        parts = [(res.results[2 * b + h]["ot"], res.results[2 * b + h]["r"])
                 for h in range(2)]
        out[:, b] = combine_host(parts)
    if _trace:
        _CACHE["last_result"] = res
    return out



# revision 5
# speedup vs baseline: 1.2420x; 1.2420x over previous
"""Complex-valued dot-product attention (B=4, S=4096, D=64) on 8 TRN2 cores.

Harness entry: kernel(**inputs) -> np.ndarray [2, 4, 4096, 64] fp32.

Math (per batch): s = (q_re + i q_im)(k_re + i k_im)^T / 8,
w = softmax(|s|, axis=keys), out = stack(w @ v_re, w @ v_im).

Sharding: core c = (batch b = c//2, key half h = c%2). Each core computes a
partial softmax (flash-style) over its 2048 keys for all 4096 queries:
  OT = sum_k E^T [v_re|v_im],  R = sum_k E^T,  E = exp(|s|/8 - 6)
and the gather step combines o = (OT_0 + OT_1) / (R_0 + R_1) exactly (the
-6 bias is a shared constant so no per-shard max bookkeeping is needed).

Device kernel (per core): scores are built transposed, s^T [k_part, q_free],
via fp16 matmuls contracting 128 re/im-concatenated channels. Per k-tile a
custom fused DVE op (out = in0^2 + in1^2) drains both PSUM banks into
|s_raw|^2 in fp16. The whole  sqrt -> exp  chain then runs as a SINGLE
scalar-engine pass: the kernel compiles against a patched PWP activation
table in which the `exp` buckets are rewritten (same expansion points,
Taylor coefficients of g(t) = exp(sqrt(t) - 6)), so one ACT instruction
computes E = exp(|s_raw|/8 - 6) directly from |s_raw|^2 with scale=1/64.
The softmax row-sum (ones-matmul) and PV matmuls stream E^T back through
the tensor engine, PSUM-accumulated across the 16 k-tiles.
"""

import hashlib
import json
import os
import shutil

import numpy as np

import concourse.bacc as bacc
import concourse.bass as bass
import concourse.mybir as mybir
import concourse.tile as tile

F16 = mybir.dt.float16
F32 = mybir.dt.float32
AF = mybir.ActivationFunctionType

D = 64
SCALE = 1.0 / np.sqrt(np.float32(D))
EXP_BIAS = -6.0
M2_FLOOR = 2.0 ** -10  # keeps table input off the small-signal path

_OPS = {}
_CACHE = {}


# ------------------------------------------------------- custom DVE op
def _register(name, make_spec):
    from concourse import dve_ops
    from concourse.dve_ops import DveOp
    from concourse.dve_spec import lower, _has_src1
    from concourse.dve_uop import DveOpSpec

    if name in _OPS:
        return _OPS[name]
    for op in dve_ops.OPS:
        if op.name == name:
            _OPS[name] = op
            return op
    spec = make_spec()
    row = dve_ops._CUSTOM_DVE_ROW_BASE + len(dve_ops.OPS)
    assert row < 0x20
    dve_ops._SUB_OPCODE_FOR_NAME[name] = row
    shas = {}
    for ver in ("v3", "v4"):
        ds = DveOpSpec(
            name=name, opcode=row, uops=lower(spec, ver=ver), rd1_en=_has_src1(spec)
        )
        shas[ver] = ds.sha(ver)
    op = DveOp(name, spec, subdim=False, uops_sha=shas)
    dve_ops.OPS.append(op)
    dve_ops.CUSTOM_DVE_SPECS[name] = spec
    _OPS[name] = op
    return op


def register_sumsq():
    from concourse.dve_spec import Spec, Src0, Src1, sq

    return _register(
        "SUMSQ_ANT",
        lambda: Spec(
            body=sq(Src0) + Src1,
            reference=lambda in0, in1, s0, s1, imm2: (
                in0.astype(np.float32) ** 2 + in1.astype(np.float32)
            ),
        ),
    )


def register_sqdrain():
    from concourse.dve_spec import Spec, Src0, sq

    return _register(
        "SQDRAIN_ANT",
        lambda: Spec(
            body=sq(Src0),
            reference=lambda in0, in1, s0, s1, imm2: (
                in0.astype(np.float32) ** 2
            ),
        ),
    )


# ------------------------------------------------- patched activation table
# Rewrites every `exp` bucket (identified by its Taylor signature
# d0=e^a, d1=e^a, d2=e^a/2) so the scalar engine's Exp computes
# g(t) = exp(sqrt(t) - 6) for t > 0 and e^-6 for t <= 0.
def _patch_exp_buckets(raw):
    arr = np.frombuffer(raw, dtype=np.float32).reshape(-1, 8).copy()
    a = arr[:, 4].astype(np.float64)
    d0 = arr[:, 0].astype(np.float64)
    d1 = arr[:, 1].astype(np.float64)
    d2 = arr[:, 2].astype(np.float64)
    with np.errstate(over="ignore", invalid="ignore"):
        ea = np.exp(a)
        is_exp = (
            np.isfinite(ea)
            & (d0 > 0)
            & (np.abs(d0 - ea) <= 1e-5 * ea)
            & (np.abs(d1 - ea) <= 1e-5 * ea)
            & (np.abs(2.0 * d2 - ea) <= 1e-4 * ea)
        )
    pos = is_exp & (a > 0)
    A = a[pos]
    sA = np.sqrt(A)
    E = np.exp(sA + EXP_BIAS)
    arr[pos, 0] = E.astype(np.float32)
    arr[pos, 1] = (E / (2 * sA)).astype(np.float32)
    arr[pos, 2] = ((E * (1 / (4 * A) - 1 / (4 * A**1.5))) / 2.0).astype(np.float32)
    arr[pos, 3] = (
        (E * (1 / (8 * A**1.5) - 3 / (8 * A**2) + 3 / (8 * A**2.5))) / 6.0
    ).astype(np.float32)
    neg = is_exp & (a <= 0)
    arr[neg, 0] = np.float32(np.exp(EXP_BIAS))
    arr[neg, 1] = 0.0
    arr[neg, 2] = 0.0
    arr[neg, 3] = 0.0
    return arr.tobytes(), int(pos.sum())


def _build_table_dir():
    """Copy the compiler's default PWP table dir, patching exp buckets.

    Returns (table_dir, short content tag). Idempotent per content tag.
    """
    from neuronxcc.driver.Job import Job
    from neuronxcc.driver.jobs.support.FindActInfo import findActInfoFile

    src_json = os.environ.get("BASS_ACT_ROOT_JSON_PATH") or findActInfoFile(
        Job.getPackageDir(), "core_v4"
    )
    src = os.path.dirname(src_json)
    with open(src_json) as f:
        info = json.load(f)

    h = hashlib.sha256(b"expsqrt_v2")
    patches = {}
    n_sets = 0
    for s in info["act_func_sets"]:
        if "exp" not in s.get("act", {}):
            continue
        p = os.path.join(src, s["bkt_bin"])
        with open(p, "rb") as f:
            raw = f.read()
        patched, n_pos = _patch_exp_buckets(raw)
        assert n_pos > 100, f"{s['name']}: only {n_pos} exp buckets matched"
        patches[s["bkt_bin"]] = patched
        h.update(patched)
        n_sets += 1
    assert n_sets >= 1, "no exp-containing activation table sets found"
    tag = h.hexdigest()[:10]

    dst = f"/tmp/acttab_{tag}"
    if not os.path.exists(os.path.join(dst, os.path.basename(src_json))):
        tmp = dst + f".tmp{os.getpid()}"
        if os.path.exists(tmp):
            shutil.rmtree(tmp)
        shutil.copytree(src, tmp)
        os.chmod(tmp, 0o755)
        for fn in os.listdir(tmp):
            os.chmod(os.path.join(tmp, fn), 0o644)
        for fn, data in patches.items():
            with open(os.path.join(tmp, fn), "wb") as f:
                f.write(data)
        if os.path.exists(dst):
            shutil.rmtree(tmp)
        else:
            os.rename(tmp, dst)
    return os.path.join(dst, os.path.basename(src_json)), tag


# ---------------------------------------------------------------- device kernel
def build_kernel(SQ=4096, SK=2048, CH=512, dve_drain_mod=4):
    """dve_drain_mod: im^2 PSUM drains go to DVE (instead of ACT Square)
    for k-tiles with kt % dve_drain_mod == dve_drain_mod - 1. Balances the
    scalar and vector engines; dual-PSUM DVE reads are illegal so the
    re^2+im^2 combine always reads re from PSUM and im^2 from SBUF."""
    table_json, tag = _build_table_dir()
    os.environ["BASS_ACT_ROOT_JSON_PATH"] = table_json

    sumsq = register_sumsq()
    sqdrain = register_sqdrain()
    KT = SK // 128
    NITER = SQ // (2 * CH)
    W = 2 * CH
    REG = KT * W

    nc = bacc.Bacc("TRN2", target_bir_lowering=False)
    # Tensor names carry the table tag: the PJRT compile cache is keyed on
    # the BIR, and the activation table (env var) is not part of that key.
    qc_d = nc.dram_tensor(f"qc_{tag}", [128, SQ], F16, kind="ExternalInput")
    kc1_d = nc.dram_tensor(f"kc1_{tag}", [128, SK], F16, kind="ExternalInput")
    kc2_d = nc.dram_tensor(f"kc2_{tag}", [128, SK], F16, kind="ExternalInput")
    vc_d = nc.dram_tensor(f"vc_{tag}", [KT, 128, 128], F16, kind="ExternalInput")
    ot_d = nc.dram_tensor(f"ot_{tag}", [128, SQ], F32, kind="ExternalOutput")
    r_d = nc.dram_tensor(f"r_{tag}", [1, SQ], F32, kind="ExternalOutput")

    with tile.TileContext(nc) as tc:
        with (
            tc.tile_pool(name="singles", bufs=1) as singles,
            tc.tile_pool(name="stage", bufs=2) as stage,
            tc.tile_pool(name="outp", bufs=2) as outp,
            tc.tile_pool(name="ps_re", bufs=2, space="PSUM") as ps_re_pool,
            tc.tile_pool(name="ps_im", bufs=2, space="PSUM") as ps_im_pool,
            tc.tile_pool(name="ps_acc", bufs=1, space="PSUM") as ps_acc,
        ):
            qc = singles.tile([128, SQ], F16)
            kc1 = singles.tile([128, SK], F16)
            kc2 = singles.tile([128, SK], F16)
            vc = singles.tile([128, KT * 128], F16)
            ones = singles.tile([128, 1], F16)
            for i in range(4):
                ksl = slice(i * (SK // 4), (i + 1) * (SK // 4))
                nc.sync.dma_start(kc1[:, ksl], kc1_d.ap()[:, ksl])
                nc.sync.dma_start(kc2[:, ksl], kc2_d.ap()[:, ksl])
                qsl = slice(i * (SQ // 4), (i + 1) * (SQ // 4))
                nc.sync.dma_start(qc[:, qsl], qc_d.ap()[:, qsl])
            for kt in range(KT):
                nc.sync.dma_start(vc[:, kt * 128 : (kt + 1) * 128], vc_d.ap()[kt])
            nc.any.memset(ones[:], 1.0)
            g_bias = singles.tile([128, 1], F32)
            nc.any.memset(g_bias[:], M2_FLOOR)

            state = {}

            def emit_a_kt(qi, kt, v_buf):
                k_sl = slice(kt * 128, (kt + 1) * 128)
                res, ims = [], []
                for half in range(2):
                    res.append(ps_re_pool.tile([128, CH], F32, tag="re",
                                               name=f"re_{qi}_{kt}_{half}"))
                    ims.append(ps_im_pool.tile([128, CH], F32, tag="im",
                                               name=f"im_{qi}_{kt}_{half}"))
                for half in range(2):
                    q0 = qi * W + half * CH
                    nc.tensor.matmul(ims[half][:], kc2[:, k_sl],
                                     qc[:, q0 : q0 + CH], start=True, stop=True)
                for half in range(2):
                    q0 = qi * W + half * CH
                    nc.tensor.matmul(res[half][:], kc1[:, k_sl],
                                     qc[:, q0 : q0 + CH], start=True, stop=True)
                dve_drain = (kt % dve_drain_mod) == dve_drain_mod - 1
                for half in range(2):
                    reg = slice(kt * W + half * CH, kt * W + (half + 1) * CH)
                    if dve_drain:
                        nc.vector._custom_dve(
                            sqdrain, out=v_buf[:, reg], in0=ims[half][:]
                        )
                    else:
                        nc.scalar.activation(v_buf[:, reg], ims[half][:], AF.Square)
                    nc.vector._custom_dve(
                        sumsq, out=v_buf[:, reg], in0=res[half][:], in1=v_buf[:, reg]
                    )

            def emit_g_pair(qi, kt0):
                # E = exp(sqrt(m2/64 + floor) - 6) via the patched exp table
                v_buf = state[qi]["v_buf"]
                sl = slice(kt0 * W, (kt0 + 2) * W)
                nc.scalar.activation(
                    v_buf[:, sl], v_buf[:, sl], AF.Exp,
                    scale=float(SCALE * SCALE), bias=g_bias[:],
                )

            def emit_d_kt(qi, kt):
                st = state[qi]
                if "ps_o" not in st:
                    st["ps_o"] = [
                        ps_acc.tile([128, CH], F32, tag="oA", name=f"ps_oA_{qi}"),
                        ps_acc.tile([128, CH], F32, tag="oB", name=f"ps_oB_{qi}"),
                    ]
                    st["ps_r"] = [
                        ps_acc.tile([1, CH], F32, tag="rA", name=f"ps_rA_{qi}"),
                        ps_acc.tile([1, CH], F32, tag="rB", name=f"ps_rB_{qi}"),
                    ]
                v_buf = st["v_buf"]
                for half in range(2):
                    e_half = v_buf[:, kt * W + half * CH : kt * W + (half + 1) * CH]
                    nc.tensor.matmul(
                        st["ps_r"][half][:], ones[:], e_half,
                        start=(kt == 0), stop=(kt == KT - 1),
                    )
                    nc.tensor.matmul(
                        st["ps_o"][half][:],
                        vc[:, kt * 128 : (kt + 1) * 128], e_half,
                        start=(kt == 0), stop=(kt == KT - 1),
                    )

            def emit_out(qi):
                st = state.pop(qi)
                o_sb = outp.tile([128, W], F32, tag="o_sb", name=f"osb{qi}")
                r_sb = outp.tile([1, W], F32, tag="r_sb", name=f"rsb{qi}")
                nc.vector.tensor_copy(o_sb[:, 0:CH], st["ps_o"][0][:])
                nc.vector.tensor_copy(o_sb[:, CH : 2 * CH], st["ps_o"][1][:])
                nc.vector.tensor_copy(r_sb[:, 0:CH], st["ps_r"][0][:])
                nc.vector.tensor_copy(r_sb[:, CH : 2 * CH], st["ps_r"][1][:])
                nc.sync.dma_start(ot_d.ap()[:, qi * W : (qi + 1) * W], o_sb[:])
                nc.sync.dma_start(r_d.ap()[:, qi * W : (qi + 1) * W], r_sb[:])

            for qi in range(NITER + 1):
                prev = qi - 1

                if qi < NITER:
                    v_buf = stage.tile([128, REG], F16, tag="v_buf",
                                       name=f"vbuf_{qi}")
                    state[qi] = {"v_buf": v_buf}
                    for kt in range(KT):
                        emit_a_kt(qi, kt, v_buf)
                        if prev >= 0:
                            if kt % 2 == 0:
                                emit_g_pair(prev, kt)
                            elif kt >= 3:
                                emit_d_kt(prev, kt - 3)
                                emit_d_kt(prev, kt - 2)
                    if prev >= 0:
                        for dk in range(KT - 2, KT):
                            emit_d_kt(prev, dk)
                        emit_out(prev)
                else:
                    for kt0 in range(0, KT, 2):
                        emit_g_pair(prev, kt0)
                        emit_d_kt(prev, kt0)
                        emit_d_kt(prev, kt0 + 1)
                    emit_out(prev)

    nc.compile()
    _CACHE["names"] = {
        "qc": f"qc_{tag}", "kc1": f"kc1_{tag}", "kc2": f"kc2_{tag}",
        "vc": f"vc_{tag}", "ot": f"ot_{tag}", "r": f"r_{tag}",
    }
    return nc


# ---------------------------------------------------------------- host packing
def pack_core(q_re, q_im, k_re, k_im, v_re, v_im):
    SK = k_re.shape[0]
    KT = SK // 128
    n = _CACHE["names"]
    qc = np.concatenate([q_re.T, q_im.T], axis=0).astype(np.float16)
    kc1 = np.concatenate([k_re.T, -k_im.T], axis=0).astype(np.float16)
    kc2 = np.concatenate([k_im.T, k_re.T], axis=0).astype(np.float16)
    vc = np.concatenate([v_re, v_im], axis=1).astype(np.float16).reshape(KT, 128, 128)
    return {n["qc"]: np.ascontiguousarray(qc), n["kc1"]: np.ascontiguousarray(kc1),
            n["kc2"]: np.ascontiguousarray(kc2), n["vc"]: np.ascontiguousarray(vc)}


def combine_host(parts):
    ot = sum(p[0].astype(np.float64) for p in parts)
    r = sum(p[1].reshape(-1).astype(np.float64) for p in parts)
    o = (ot / r[None, :]).astype(np.float32)
    return np.stack([o[0:D].T, o[D : 2 * D].T], axis=0)


# ---------------------------------------------------------------- harness entry
B, S = 4, 4096
SK_HALF = 2048


def _get_nc():
    if "nc" not in _CACHE:
        _CACHE["nc"] = build_kernel()
    return _CACHE["nc"]


def kernel(q_re, q_im, k_re, k_im, v_re, v_im, _trace=False):
    from concourse import bass_utils

    arrs = [np.asarray(a, dtype=np.float32)
            for a in (q_re, q_im, k_re, k_im, v_re, v_im)]
    assert arrs[0].shape == (B, S, D)

    nc = _get_nc()
    names = _CACHE["names"]
    maps = []
    for c in range(8):
        b, h = c // 2, c % 2
        ks = slice(h * SK_HALF, (h + 1) * SK_HALF)
        maps.append(pack_core(
            arrs[0][b], arrs[1][b],
            arrs[2][b, ks], arrs[3][b, ks],
            arrs[4][b, ks], arrs[5][b, ks]))
    res = None
    last_exc = None
    for attempt in range(3):
        try:
            res = bass_utils.run_bass_kernel_spmd(
                nc, maps, core_ids=list(range(8)), trace=_trace)
            break
        except Exception as e:  # transient device wedge: retry untraced
            last_exc = e
            _trace = False
    if res is None:
        raise last_exc
    out = np.empty((2, B, S, D), dtype=np.float32)
    for b in range(B):
        parts = [(res.results[2 * b + h][names["ot"]],
                  res.results[2 * b + h][names["r"]])
                 for h in range(2)]
        out[:, b] = combine_host(parts)
    if _trace:
        _CACHE["last_result"] = res
    return out
